# revision 6
# baseline (speedup 1.0000x reference)
"""Causal flash attention (B=2, S=2048, H=16, D=128, fp32) on 8 Trainium2 cores.

Sharding: the 32 (b,h) pairs are split 4-per-core (data + head parallel);
attention is embarrassingly parallel over (b,h), so the SPMD program is
identical on every core and needs no collectives.

Per-core kernel layout ("flipped" orientation):
  - scores are computed transposed: S^T[j, i] = sum_d K[j,d] Q[i,d], with the
    key position j on PSUM partitions and query position i on the free axis.
    lhsT = K^T tile [d, j-block], rhs = Q^T [d, i] (both produced by PE
    transposes of the naturally-loaded tiles).
  - softmax needs no max subtraction (scores ~ N(0,1), exp is safe in fp32);
    exp runs on the scalar engine with the 1/sqrt(D) scale folded in, writing
    P^T straight from PSUM to SBUF.  Causal masking is only needed on the
    diagonal 128x128 blocks (affine_select) -- strictly-upper j-blocks are
    never computed.
  - PV needs no transpose of P: O^T[d, i] = sum_j V[j,d] P^T[j,i] with
    lhsT = V tile in natural layout and rhs = P^T, accumulated over j-blocks
    in PSUM.  The softmax denominator comes from ones^T @ P^T matmuls.
  - O^T is copied to SBUF, transposed back 128-block-wise on the PE, and
    normalized during the PSUM->SBUF staging copy with per-partition
    reciprocal denominators (obtained by tiny PE transposes of the fp32
    reciprocal row), then DMA'd out.

Matmuls run in float32r (tf32-like) for 4x throughput over fp32; all
float32r operands are produced (rounded) by DVE/ACT writes as the BIR
verifier requires.
"""

import math
from contextlib import ExitStack

import numpy as np

import concourse.bass as bass
import concourse.tile as tile
from concourse import bacc, mybir
from concourse.bass_utils import run_bass_kernel_spmd
from concourse.masks import make_identity

B, S, H, D = 2, 2048, 16, 128
NCORES = 8
NPAIRS = B * H          # 32 (b,h) pairs
PPC = NPAIRS // NCORES  # 4 pairs per core
SCALE = 1.0 / math.sqrt(D)
FP32 = mybir.dt.float32
FP32R = mybir.dt.float32r
NB = S // 128           # 16 key blocks (128 wide)
NCH = S // 512          # 4 query chunks (512 wide)

# P^T storage: for key-block jb we keep query columns i in [512*(jb//4), S)
PT_W = [S - 512 * (jb // 4) for jb in range(NB)]
PT_OFF = np.cumsum([0] + PT_W).tolist()
PT_COLS = PT_OFF[-1]    # 20480 columns (80KB/partition)


def _emit_pair(nc, pools, io, p):
    """Emit one (b,h) pair's attention."""
    q, k, v, o = io
    consts, qkv, nat, ptp, onp, rdp, psum = pools
    ident, ones_col = consts

    # ---- Phase A: load Q,K natural tiles, PE-transpose into [d, s]; load V.
    qt = qkv.tile([128, S], FP32R, tag="qt")   # Q^T: d on partitions
    kt = qkv.tile([128, S], FP32R, tag="kt")   # K^T: d on partitions
    vt = qkv.tile([128, NB, 128], FP32, tag="vt")   # V natural: j on partitions
    vtr = qkv.tile([128, NB, 128], FP32R, tag="vtr")
    nc.sync.dma_start(out=vt, in_=v[p].rearrange("(jb j) d -> j jb d", j=128))
    nc.vector.tensor_copy(out=vtr.rearrange("j a b -> j (a b)"),
                          in_=vt.rearrange("j a b -> j (a b)"))
    for src, dst in ((q, qt), (k, kt)):
        grp = src[p].rearrange("(g t s) d -> g s t d", g=4, t=4, s=128)
        for g in range(4):
            natt = nat.tile([128, 4, 128], FP32, tag="nat", name=f"nat_{p}_{g}")
            nc.sync.dma_start(out=natt, in_=grp[g])
            for t in range(4):
                tb = 4 * g + t
                tp = psum.tile([128, 128], FP32, tag="st", bufs=2,
                               name=f"tpose_{p}_{g}_{t}")
                nc.tensor.transpose(tp, natt[:, t, :], ident)
                nc.vector.tensor_copy(out=dst[:, 128 * tb:128 * (tb + 1)], in_=tp)

    # ---- Phase B: S^T = K_jb @ Q^T (fp32r), exp on ACT, diag causal mask.
    pt = ptp.tile([128, PT_COLS], FP32R, tag="pt")
    for jb in range(NB):
        st0 = 512 * (jb // 4)        # first stored global column
        r = 128 * (jb % 4)           # computed start, relative to st0
        wj = S - st0                 # stored width
        for t in range((wj + 1023) // 1024):
            a = 1024 * t             # tile start, relative to st0
            b_ = min(a + 1024, wj)
            lo = r if t == 0 else a
            st = psum.tile([128, 1024], FP32, tag="st", bufs=2,
                           name=f"st_{p}_{jb}_{t}")
            p0 = lo
            while p0 < b_:
                p1 = min((p0 // 512 + 1) * 512, b_)
                nc.tensor.matmul(
                    out=st[:, p0 - a:p1 - a],
                    lhsT=kt[:, 128 * jb:128 * (jb + 1)],
                    rhs=qt[:, st0 + p0:st0 + p1],
                    start=True, stop=True)
                p0 = p1
            nc.scalar.activation(
                out=pt[:, PT_OFF[jb] + lo:PT_OFF[jb] + b_],
                in_=st[:, lo - a:b_ - a],
                func=mybir.ActivationFunctionType.Exp,
                scale=SCALE)
        # causal mask on the diagonal block: keep i_local >= j_local
        dg = pt[:, PT_OFF[jb] + r:PT_OFF[jb] + r + 128]
        nc.gpsimd.affine_select(
            out=dg, in_=dg,
            compare_op=mybir.AluOpType.is_ge,
            fill=0.0, base=0,
            pattern=[[1, 128]], channel_multiplier=-1)

    # ---- Phase C: per query chunk: denominator, PV accumulate, transpose
    # back, normalize during staging, store.
    for c in range(NCH):
        njb = 4 * c + 4              # key blocks feeding this chunk

        def pt_slice(jb):
            m = c - jb // 4          # stored-relative 512-block index
            rr = 128 * (jb % 4) if m == 0 else 0
            base = PT_OFF[jb] + 512 * m
            return rr, pt[:, base + rr:base + 512]

        den = psum.tile([1, 512], FP32, tag="msc", bufs=2, name=f"den_{p}_{c}")
        for jb in range(njb):
            rr, sl = pt_slice(jb)
            nc.tensor.matmul(out=den[:, rr:512], lhsT=ones_col, rhs=sl,
                             start=(jb == 0), stop=(jb == njb - 1))
        rd = rdp.tile([1, 512], FP32, tag="rd", name=f"rd_{p}_{c}")
        nc.vector.reciprocal(out=rd, in_=den)

        ot = psum.tile([128, 512], FP32, tag="ot", bufs=2, name=f"ot_{p}_{c}")
        for jb in range(njb):
            rr, sl = pt_slice(jb)
            nc.tensor.matmul(out=ot[:, rr:512], lhsT=vtr[:, jb, :], rhs=sl,
                             start=(jb == 0), stop=(jb == njb - 1))
        on = onp.tile([128, 512], FP32, tag="on", name=f"on_{p}_{c}")
        nc.vector.tensor_copy(out=on, in_=ot)

        stg = onp.tile([128, 4, 128], FP32, tag="stg", name=f"stg_{p}_{c}")
        for bb in range(4):
            tu = psum.tile([128, 128], FP32, tag="ot", bufs=2,
                           name=f"tu_{p}_{c}_{bb}")
            nc.tensor.transpose(tu, on[:, 128 * bb:128 * (bb + 1)], ident)
            rdt = psum.tile([128, 1], FP32, tag="msc", bufs=2,
                            name=f"rdt_{p}_{c}_{bb}")
            nc.tensor.transpose(rdt, rd[:, 128 * bb:128 * (bb + 1)],
                                ident[0:1, 0:1])
            rds = rdp.tile([128, 1], FP32, tag="rds", name=f"rds_{p}_{c}_{bb}")
            nc.vector.tensor_copy(out=rds, in_=rdt)
            nc.vector.tensor_scalar_mul(stg[:, bb, :], tu, rds)
        oview = o[p].rearrange("(c4 bb i) d -> c4 i bb d", c4=NCH, bb=4, i=128)
        nc.sync.dma_start(out=oview[c], in_=stg)


def _emit(ctx, tc, o, q, k, v):
    nc = tc.nc
    consts = ctx.enter_context(tc.tile_pool(name="consts", bufs=1))
    ident = consts.tile([128, 128], FP32)
    make_identity(nc, ident)
    ones_f32 = consts.tile([128, 1], FP32)
    nc.vector.memset(ones_f32, 1.0)
    ones_col = consts.tile([128, 1], FP32R)
    nc.vector.tensor_copy(out=ones_col, in_=ones_f32)

    qkv = ctx.enter_context(tc.tile_pool(name="qkv", bufs=2))
    nat = ctx.enter_context(tc.tile_pool(name="nat", bufs=4))
    ptp = ctx.enter_context(tc.tile_pool(name="ptp", bufs=1))
    onp = ctx.enter_context(tc.tile_pool(name="onp", bufs=2))
    rdp = ctx.enter_context(tc.tile_pool(name="rdp", bufs=4))
    psum = ctx.enter_context(tc.tile_pool(name="psum", bufs=2, space="PSUM"))

    pools = ((ident, ones_col), qkv, nat, ptp, onp, rdp, psum)
    for p in range(PPC):
        _emit_pair(nc, pools, (q, k, v, o), p)


_PROGRAM = None


def _build_program():
    global _PROGRAM
    if _PROGRAM is not None:
        return _PROGRAM
    nc = bacc.Bacc("TRN2", target_bir_lowering=False, debug=False)
    q = nc.dram_tensor("q", [PPC, S, D], FP32, kind="ExternalInput").ap()
    k = nc.dram_tensor("k", [PPC, S, D], FP32, kind="ExternalInput").ap()
    v = nc.dram_tensor("v", [PPC, S, D], FP32, kind="ExternalInput").ap()
    o = nc.dram_tensor("o", [PPC, S, D], FP32, kind="ExternalOutput").ap()
    with tile.TileContext(nc) as tc:
        with ExitStack() as ctx:
            _emit(ctx, tc, o, q, k, v)
    nc.compile()
    _PROGRAM = nc
    return nc


def _shard(x):
    """[B, S, H, D] -> list of NCORES arrays [PPC, S, D] ((b,h)-major)."""
    xt = np.ascontiguousarray(
        np.transpose(np.asarray(x, dtype=np.float32), (0, 2, 1, 3))
    ).reshape(NPAIRS, S, D)
    return [xt[PPC * c:PPC * (c + 1)] for c in range(NCORES)]


def run_sharded(q, k, v, **spmd_kwargs):
    """Run the SPMD program; returns BassKernelResults."""
    nc = _build_program()
    qs, ks, vs = _shard(q), _shard(k), _shard(v)
    in_maps = [{"q": qs[c], "k": ks[c], "v": vs[c]} for c in range(NCORES)]
    res = run_bass_kernel_spmd(nc, in_maps, list(range(NCORES)), **spmd_kwargs)
    return res


def kernel(q, k, v):
    res = run_sharded(q, k, v)
    full = np.concatenate([res.results[c]["o"] for c in range(NCORES)], axis=0)
    out = full.reshape(B, H, S, D).transpose(0, 2, 1, 3)
    return np.ascontiguousarray(out)


# revision 7
# speedup vs baseline: 1.0026x; 1.0026x over previous
"""Causal flash attention (B=2, S=2048, H=16, D=128, fp32) on 8 Trainium2 cores.

Sharding: the 32 (b,h) pairs are split 4-per-core (data + head parallel);
attention is embarrassingly parallel over (b,h), so the SPMD program is
identical on every core and needs no collectives.

Per-core kernel layout ("flipped" orientation):
  - scores are computed transposed: S^T[j, i] = sum_d K[j,d] Q[i,d], with the
    key position j on PSUM partitions and query position i on the free axis.
    lhsT = K^T tile [d, j-block], rhs = Q^T [d, i] (both produced by PE
    transposes of the naturally-loaded tiles).
  - softmax needs no max subtraction (scores ~ N(0,1), exp is safe in fp32);
    exp runs on the scalar engine with the 1/sqrt(D) scale folded in, writing
    P^T straight from PSUM to SBUF.  Causal masking is only needed on the
    diagonal 128x128 blocks (affine_select) -- strictly-upper j-blocks are
    never computed.
  - PV needs no transpose of P: O^T[d, i] = sum_j V[j,d] P^T[j,i] with
    lhsT = V tile in natural layout and rhs = P^T, accumulated over j-blocks
    in PSUM.  The softmax denominator comes from ones^T @ P^T matmuls.
  - O^T is copied to SBUF, transposed back 128-block-wise on the PE, and
    normalized during the PSUM->SBUF staging copy with per-partition
    reciprocal denominators (obtained by tiny PE transposes of the fp32
    reciprocal row), then DMA'd out.

Matmuls run in float32r (tf32-like) for 4x throughput over fp32; all
float32r operands are produced (rounded) by DVE/ACT writes as the BIR
verifier requires.
"""

import math
from contextlib import ExitStack

import numpy as np

import concourse.bass as bass
import concourse.tile as tile
from concourse import bacc, mybir
from concourse.bass_utils import run_bass_kernel_spmd
from concourse.masks import make_identity

B, S, H, D = 2, 2048, 16, 128
NCORES = 8
NPAIRS = B * H          # 32 (b,h) pairs
PPC = NPAIRS // NCORES  # 4 pairs per core
SCALE = 1.0 / math.sqrt(D)
FP32 = mybir.dt.float32
FP32R = mybir.dt.float32r
NB = S // 128           # 16 key blocks (128 wide)
NCH = S // 512          # 4 query chunks (512 wide)

# P^T storage: for key-block jb we keep query columns i in [512*(jb//4), S)
PT_W = [S - 512 * (jb // 4) for jb in range(NB)]
PT_OFF = np.cumsum([0] + PT_W).tolist()
PT_COLS = PT_OFF[-1]    # 20480 columns (80KB/partition)


def _emit_pair(nc, pools, io, p):
    """Emit one (b,h) pair's attention."""
    q, k, v, o = io
    consts, qkv, nat, ptp, onp, rdp, psum = pools
    ident, ones_col = consts

    # ---- Phase A: load Q,K natural tiles, PE-transpose into [d, s]; load V.
    qt = qkv.tile([128, S], FP32R, tag="qt")   # Q^T: d on partitions
    kt = qkv.tile([128, S], FP32R, tag="kt")   # K^T: d on partitions
    vt = qkv.tile([128, NB, 128], FP32, tag="vt")   # V natural: j on partitions
    vtr = qkv.tile([128, NB, 128], FP32R, tag="vtr")
    nc.sync.dma_start(out=vt, in_=v[p].rearrange("(jb j) d -> j jb d", j=128))
    nc.vector.tensor_copy(out=vtr.rearrange("j a b -> j (a b)"),
                          in_=vt.rearrange("j a b -> j (a b)"))
    for src, dst in ((q, qt), (k, kt)):
        grp = src[p].rearrange("(g t s) d -> g s t d", g=4, t=4, s=128)
        for g in range(4):
            natt = nat.tile([128, 4, 128], FP32, tag="nat", name=f"nat_{p}_{g}")
            nc.sync.dma_start(out=natt, in_=grp[g])
            for t in range(4):
                tb = 4 * g + t
                tp = psum.tile([128, 128], FP32, tag="st", bufs=2,
                               name=f"tpose_{p}_{g}_{t}")
                nc.tensor.transpose(tp, natt[:, t, :], ident)
                nc.vector.tensor_copy(out=dst[:, 128 * tb:128 * (tb + 1)], in_=tp)

    # ---- Phase B+C interleaved: after every 4th key block's exp, the PE has
    # everything it needs for query chunk c = jb//4 -- emit its denominator
    # and PV accumulation immediately so the PE never drains while ACT works
    # through the exps.  The output stage (transposes + normalize + DMA) for
    # chunk c is deferred until after chunk c+1's matmuls so the PE never
    # waits on a freshly produced DVE result.
    pt = ptp.tile([128, PT_COLS], FP32R, tag="pt")
    oview = o[p].rearrange("(c4 bb i) d -> c4 i bb d", c4=NCH, bb=4, i=128)

    def pt_slice(c, jb):
        m = c - jb // 4              # stored-relative 512-block index
        rr = 128 * (jb % 4) if m == 0 else 0
        base = PT_OFF[jb] + 512 * m
        return rr, pt[:, base + rr:base + 512]

    def emit_chunk(c):
        """Denominator + PV accumulation for query chunk c."""
        njb = 4 * c + 4
        den = psum.tile([1, 512], FP32, tag="msc", bufs=2, name=f"den_{p}_{c}")
        for jb in range(njb):
            rr, sl = pt_slice(c, jb)
            nc.tensor.matmul(out=den[:, rr:512], lhsT=ones_col, rhs=sl,
                             start=(jb == 0), stop=(jb == njb - 1))
        rd = rdp.tile([1, 512], FP32, tag="rd", name=f"rd_{p}_{c}")
        nc.vector.reciprocal(out=rd, in_=den)
        ot = psum.tile([128, 512], FP32, tag="ot", bufs=2, name=f"ot_{p}_{c}")
        for jb in range(njb):
            rr, sl = pt_slice(c, jb)
            nc.tensor.matmul(out=ot[:, rr:512], lhsT=vtr[:, jb, :], rhs=sl,
                             start=(jb == 0), stop=(jb == njb - 1))
        on = onp.tile([128, 512], FP32, tag="on", name=f"on_{p}_{c}")
        nc.vector.tensor_copy(out=on, in_=ot)
        return rd, on

    def emit_output(c, rd, on):
        """Transpose back, normalize, and store query chunk c."""
        stg = onp.tile([128, 4, 128], FP32, tag="stg", name=f"stg_{p}_{c}")
        for bb in range(4):
            tu = psum.tile([128, 128], FP32, tag="ot", bufs=2,
                           name=f"tu_{p}_{c}_{bb}")
            nc.tensor.transpose(tu, on[:, 128 * bb:128 * (bb + 1)], ident)
            rdt = psum.tile([128, 1], FP32, tag="msc", bufs=2,
                            name=f"rdt_{p}_{c}_{bb}")
            nc.tensor.transpose(rdt, rd[:, 128 * bb:128 * (bb + 1)],
                                ident[0:1, 0:1])
            rds = rdp.tile([128, 1], FP32, tag="rds", name=f"rds_{p}_{c}_{bb}")
            nc.vector.tensor_copy(out=rds, in_=rdt)
            nc.vector.tensor_scalar_mul(stg[:, bb, :], tu, rds)
        nc.sync.dma_start(out=oview[c], in_=stg)

    pending = None                   # (c, rd, on) awaiting output
    for jb in range(NB):
        st0 = 512 * (jb // 4)        # first stored global column
        r = 128 * (jb % 4)           # computed start, relative to st0
        wj = S - st0                 # stored width
        for t in range((wj + 1023) // 1024):
            a = 1024 * t             # tile start, relative to st0
            b_ = min(a + 1024, wj)
            lo = r if t == 0 else a
            st = psum.tile([128, 1024], FP32, tag="st", bufs=2,
                           name=f"st_{p}_{jb}_{t}")
            p0 = lo
            while p0 < b_:
                p1 = min((p0 // 512 + 1) * 512, b_)
                nc.tensor.matmul(
                    out=st[:, p0 - a:p1 - a],
                    lhsT=kt[:, 128 * jb:128 * (jb + 1)],
                    rhs=qt[:, st0 + p0:st0 + p1],
                    start=True, stop=True)
                p0 = p1
            nc.scalar.activation(
                out=pt[:, PT_OFF[jb] + lo:PT_OFF[jb] + b_],
                in_=st[:, lo - a:b_ - a],
                func=mybir.ActivationFunctionType.Exp,
                scale=SCALE)
        # causal mask on the diagonal block: keep i_local >= j_local
        dg = pt[:, PT_OFF[jb] + r:PT_OFF[jb] + r + 128]
        nc.gpsimd.affine_select(
            out=dg, in_=dg,
            compare_op=mybir.AluOpType.is_ge,
            fill=0.0, base=0,
            pattern=[[1, 128]], channel_multiplier=-1)
        if jb % 4 == 3:
            c = jb // 4
            rd, on = emit_chunk(c)
            if pending is not None:
                emit_output(*pending)
            pending = (c, rd, on)
    emit_output(*pending)


def _emit(ctx, tc, o, q, k, v):
    nc = tc.nc
    consts = ctx.enter_context(tc.tile_pool(name="consts", bufs=1))
    ident = consts.tile([128, 128], FP32)
    make_identity(nc, ident)
    ones_f32 = consts.tile([128, 1], FP32)
    nc.vector.memset(ones_f32, 1.0)
    ones_col = consts.tile([128, 1], FP32R)
    nc.vector.tensor_copy(out=ones_col, in_=ones_f32)

    qkv = ctx.enter_context(tc.tile_pool(name="qkv", bufs=2))
    nat = ctx.enter_context(tc.tile_pool(name="nat", bufs=4))
    ptp = ctx.enter_context(tc.tile_pool(name="ptp", bufs=1))
    onp = ctx.enter_context(tc.tile_pool(name="onp", bufs=2))
    rdp = ctx.enter_context(tc.tile_pool(name="rdp", bufs=4))
    psum = ctx.enter_context(tc.tile_pool(name="psum", bufs=2, space="PSUM"))

    pools = ((ident, ones_col), qkv, nat, ptp, onp, rdp, psum)
    for p in range(PPC):
        _emit_pair(nc, pools, (q, k, v, o), p)


_PROGRAM = None


def _build_program():
    global _PROGRAM
    if _PROGRAM is not None:
        return _PROGRAM
    nc = bacc.Bacc("TRN2", target_bir_lowering=False, debug=False)
    q = nc.dram_tensor("q", [PPC, S, D], FP32, kind="ExternalInput").ap()
    k = nc.dram_tensor("k", [PPC, S, D], FP32, kind="ExternalInput").ap()
    v = nc.dram_tensor("v", [PPC, S, D], FP32, kind="ExternalInput").ap()
    o = nc.dram_tensor("o", [PPC, S, D], FP32, kind="ExternalOutput").ap()
    with tile.TileContext(nc) as tc:
        with ExitStack() as ctx:
            _emit(ctx, tc, o, q, k, v)
    nc.compile()
    _PROGRAM = nc
    return nc


def _shard(x):
    """[B, S, H, D] -> list of NCORES arrays [PPC, S, D] ((b,h)-major)."""
    xt = np.ascontiguousarray(
        np.transpose(np.asarray(x, dtype=np.float32), (0, 2, 1, 3))
    ).reshape(NPAIRS, S, D)
    return [xt[PPC * c:PPC * (c + 1)] for c in range(NCORES)]


def run_sharded(q, k, v, **spmd_kwargs):
    """Run the SPMD program; returns BassKernelResults."""
    nc = _build_program()
    qs, ks, vs = _shard(q), _shard(k), _shard(v)
    in_maps = [{"q": qs[c], "k": ks[c], "v": vs[c]} for c in range(NCORES)]
    res = run_bass_kernel_spmd(nc, in_maps, list(range(NCORES)), **spmd_kwargs)
    return res


def kernel(q, k, v):
    res = run_sharded(q, k, v)
    full = np.concatenate([res.results[c]["o"] for c in range(NCORES)], axis=0)
    out = full.reshape(B, H, S, D).transpose(0, 2, 1, 3)
    return np.ascontiguousarray(out)
